# revision 49
# baseline (speedup 1.0000x reference)
"""Multi-head attention (B=2, S=2048, D=1024, H=16) on 8 TRN2 NeuronCores.

Sharding: (batch, head-group) — core c handles batch c//4 and heads
[4*(c%4), 4*(c%4)+4). Each core projects its batch's tokens onto its 4 heads'
column-shards of Wq/Wk/Wv, runs attention for those heads, and multiplies by
its row-shard of Wo, producing a partial [S, D] output. The host sums the 4
partials per batch and adds (bo + bv @ Wo). bk is dropped entirely (a key
bias shifts every score of a query by the same constant, which softmax
cancels); bv contributes exactly bv @ Wo because attention weights sum to 1.

Device design notes:
  - All matmuls run in bf16 (1 cycle/row at any free size in the PE cost
    model); inputs and weights are cast to bf16 on the host, halving input
    DMA (22 MB -> 14 MB per core). PSUM accumulation stays f32.
  - Q/K project feature-major (out [m, s]; W stationary). V projects
    token-major (x stationary, Wv moving), giving v in [keys, dk] layout
    directly — no V transpose pass. A constant-1 column appended to each
    per-(key-chunk, head) V block produces the softmax denominator inside
    the PV matmul.
  - Scores are computed transposed (S^T [key, query]) in [128, 1024] PSUM
    tiles (2 banks, double buffered) feeding one Exp per tile on the Act
    engine — Act is the #2 engine at ~133 us busy and must never starve.
  - PV computes ctx with queries on the OUTPUT partitions: out[q, dk+1] +=
    e-slice^T @ v-chunk. Free dim is 65 instead of 512, so PV costs half
    the PE cycles of the untransposed form, and softmax normalization
    becomes a per-partition scalar op (DVE reciprocal + tensor_scalar_mul).
    Each query-chunk's 16-step PSUM accumulation is one contiguous matmul
    group — interleaving open accumulation groups in a bank miscomputes.
  - ctx [q, m] is then PE-transposed per 128x128 block into ctx^T [m, q]
    for the Wo projection (contraction over m).
  - Emission order IS each engine's execution order. The schedule runs 8
    attention units (query-half x head) paced by the exp stream, with a
    static filler table interleaving projections, PV of earlier units,
    transposes and Wo chunks between score tiles. x streams in as
    [128, 8 dc, 512] column-chunk DMAs (few DMAs; HWDGE costs ~625ns each,
    serialized). Unit 0 splits its score tiles by query-half so exp starts
    before the second half of xq arrives; unit 7 splits by query-quarter
    so most of its close-out (PV/transpose/Wo) overlaps its own exp tail.
    A 30-matmul zero warmup ramps the PE clock out of its cold p-state
    before the first DMA-gated projections.
  - PSUM: 2x1-bank pool (projections/transposes/Wo), 2x2-bank score tiles,
    2x1-bank PV accumulators ([128, 4, 65] f32). Total exactly 8 banks.

Measured (8-core run vs fp32 reference): rel err 5.4e-3 (gate 2e-2).
TimelineSim cost-model estimate: ~176.4 us per core (baseline was 254.6).
"""

import numpy as np

S = 2048          # sequence length
D = 1024          # model dim
HPC = 4           # heads per core
DK = 64           # head dim
M = HPC * DK      # per-core projection width = 256
NC = 8            # cores
IW = 1024         # attention query width per ih-half
NDC = D // 128    # 8 contraction chunks
NMC = M // 128    # 2 m-chunks (head pairs)
NKB = S // 128    # 16 key chunks
EXPW = 1024       # exp tile width
NT = 16 * IW // EXPW  # qk/exp tiles per unit = 16

_cached = {}


def _build(debug=False):
    import concourse.bass as bass
    import concourse.bacc as bacc
    import concourse.tile as tile
    import concourse.mybir as mybir
    from contextlib import ExitStack

    f32 = mybir.dt.float32
    bf16 = mybir.dt.bfloat16
    AF = mybir.ActivationFunctionType

    nc = bacc.Bacc(
        "TRN2",
        target_bir_lowering=False,
        debug=False,
        enable_asserts=False,
        num_devices=NC,
    )

    # DRAM I/O (per-core shapes)
    xqT_d = nc.dram_tensor("xqT", [D, S], bf16, kind="ExternalInput").ap()
    xkT_d = nc.dram_tensor("xkT", [D, S], bf16, kind="ExternalInput").ap()
    xvT_d = nc.dram_tensor("xvT", [D, S], bf16, kind="ExternalInput").ap()
    wq_d = nc.dram_tensor("wq", [D, M], bf16, kind="ExternalInput").ap()
    wk_d = nc.dram_tensor("wk", [D, M], bf16, kind="ExternalInput").ap()
    wv_d = nc.dram_tensor("wv", [D, M], bf16, kind="ExternalInput").ap()
    wo_d = nc.dram_tensor("wo", [M, D], bf16, kind="ExternalInput").ap()
    bq_d = nc.dram_tensor("bq", [M], f32, kind="ExternalInput").ap()
    ident_d = nc.dram_tensor("ident", [128, 128], bf16, kind="ExternalInput").ap()
    out_d = nc.dram_tensor("out", [S, D], f32, kind="ExternalOutput").ap()

    with tile.TileContext(nc) as tc:
        with ExitStack() as st:
            wp = st.enter_context(tc.tile_pool(name="wp", bufs=1))
            xt = st.enter_context(tc.tile_pool(name="xt", bufs=6))
            qkt = st.enter_context(tc.tile_pool(name="qkt", bufs=1))
            vp = st.enter_context(tc.tile_pool(name="vp", bufs=1))
            ep = st.enter_context(tc.tile_pool(name="ep", bufs=46))
            stp = st.enter_context(tc.tile_pool(name="stp", bufs=1))
            ctp = st.enter_context(tc.tile_pool(name="ctp", bufs=1))
            invp = st.enter_context(tc.tile_pool(name="invp", bufs=4))
            ostp = st.enter_context(tc.tile_pool(name="ostp", bufs=4))
            # PSUM: exactly 8 banks
            psp = st.enter_context(tc.tile_pool(name="psp", bufs=2, space="PSUM"))
            qkp = st.enter_context(tc.tile_pool(name="qkp", bufs=2, space="PSUM"))
            pvp = st.enter_context(tc.tile_pool(name="pvp", bufs=2, space="PSUM"))

            wq_sb = wp.tile([128, NDC, M], bf16, tag="wq")
            wk_sb = wp.tile([128, NDC, M], bf16, tag="wk")
            wv_sb = wp.tile([128, NDC, M], bf16, tag="wv")
            wo_sb = wp.tile([128, NMC, D], bf16, tag="wo")
            bq_sb = wp.tile([128, NMC], f32, tag="bq")
            ident = wp.tile([128, 128], bf16, tag="ident")
            qT = [qkt.tile([128, S], bf16, tag=f"qT{m}", name=f"qT{m}")
                  for m in range(NMC)]
            kT = [qkt.tile([128, S], bf16, tag=f"kT{m}", name=f"kT{m}")
                  for m in range(NMC)]
            # v in [keys, head, dk+1] layout; col DK is the constant 1
            v_sb = vp.tile([128, NKB, HPC, DK + 1], bf16, tag="v")
            stage = [[stp.tile([128, M], bf16, tag=f"st{i}{q}", name=f"st{i}{q}")
                      for q in range(8)] for i in range(2)]
            ctx_t = [[ctp.tile([128, IW], bf16, tag=f"ct{i}{m}", name=f"ct{i}{m}")
                      for m in range(NMC)] for i in range(2)]

            nc.vector.memset(v_sb[:, :, :, DK:DK + 1], 1.0)
            warm = wp.tile([128, 512], bf16, tag="warm")
            nc.vector.memset(warm, 0.0)
            wps = psp.tile([128, 512], f32, tag="ps", name="warmps")
            for _ in range(36):
                nc.tensor.matmul(wps, lhsT=warm[:, 0:128], rhs=warm,
                                 start=True, stop=True)

            w_r = lambda ap: ap.rearrange("(n p) m -> p n m", p=128)

            # ---- input DMA emission (order = SP.SEQ issue order) ----
            xc = {}   # (tensor, sh, sc) -> [128, 8, 512] tile

            def load_x(tens, dram, sh, sc):
                t = xt.tile([128, NDC, 512], bf16, tag="x",
                            name=f"x{tens}{sh}{sc}")
                nc.sync.dma_start(
                    out=t,
                    in_=dram[:, sh * 1024 + sc * 512:
                             sh * 1024 + (sc + 1) * 512].rearrange(
                                 "(n p) s -> p n s", p=128))
                xc[(tens, sh, sc)] = t

            nc.sync.dma_start(out=wk_sb, in_=w_r(wk_d))
            nc.sync.dma_start(out=wq_sb, in_=w_r(wq_d))
            load_x("k", xkT_d, 0, 0)
            load_x("q", xqT_d, 0, 0)
            nc.sync.dma_start(out=bq_sb, in_=bq_d.rearrange("(n p) -> p n", p=128))
            load_x("k", xkT_d, 0, 1)
            load_x("q", xqT_d, 0, 1)
            load_x("k", xkT_d, 1, 0)
            load_x("k", xkT_d, 1, 1)
            nc.sync.dma_start(out=wv_sb, in_=w_r(wv_d))
            load_x("v", xvT_d, 0, 0)
            load_x("v", xvT_d, 0, 1)
            load_x("v", xvT_d, 1, 0)
            load_x("v", xvT_d, 1, 1)
            load_x("q", xqT_d, 1, 0)
            load_x("q", xqT_d, 1, 1)
            nc.sync.dma_start(out=wo_sb, in_=wo_d.rearrange("(g p) n -> p g n", p=128))
            nc.sync.dma_start(out=ident, in_=ident_d)

            # ---- emission helpers ----
            def proj_mk(tens, sh, mc, sc):
                """Project q/k chunk: out [m 128, s 512]; W stationary."""
                w_sb = wq_sb if tens == "q" else wk_sb
                ps = psp.tile([128, 512], f32, tag="ps", name="ps")
                for dc in range(NDC):
                    nc.tensor.matmul(
                        ps,
                        lhsT=w_sb[:, dc, mc * 128:(mc + 1) * 128],
                        rhs=xc[(tens, sh, sc)][:, dc, :],
                        start=(dc == 0), stop=(dc == NDC - 1))
                dst = (qT if tens == "q" else kT)[mc][
                    :, sh * 1024 + sc * 512: sh * 1024 + (sc + 1) * 512]
                if tens == "q":
                    nc.vector.tensor_scalar_add(
                        out=dst, in0=ps, scalar1=bq_sb[:, mc:mc + 1])
                else:
                    nc.vector.tensor_copy(out=dst, in_=ps)

            def proj_v(kb):
                """Project v key-chunk kb: out [s 128, m 256]; x stationary."""
                sh, r = divmod(kb, 8)
                sc, q = divmod(r, 4)
                ps = psp.tile([128, 512], f32, tag="ps", name="ps")
                for dc in range(NDC):
                    nc.tensor.matmul(
                        ps[:, 0:M],
                        lhsT=xc[("v", sh, sc)][:, dc, q * 128:(q + 1) * 128],
                        rhs=wv_sb[:, dc, :],
                        start=(dc == 0), stop=(dc == NDC - 1))
                nc.vector.tensor_copy(
                    out=v_sb[:, kb, :, 0:DK],
                    in_=ps[:, 0:M].rearrange("p (a b) -> p a b", a=HPC))

            e_tiles = {}
            SPLIT_U = 7  # last unit: tiles cover [4 kb x 256 queries] so each
                         # query-quarter closes out while later quarters exp

            def qk_tile(u, t):
                """Scores^T tile [keys 128, 1024] + exp -> e (bf16)."""
                ih, h = divmod(u, HPC)
                mc, off = divmod(h, 2)
                off *= DK
                qk = qkp.tile([128, EXPW], f32, tag="qk", name="qk")
                if u == 0:
                    base, r = (0, t) if t < 8 else (4, t - 8)
                    qh, kbp = divmod(r, 4)
                    blocks = [(2 * (base + kbp) + j, qh * 512, j * 512, 512)
                              for j in range(2)]
                elif u == SPLIT_U:
                    qq, kbp = divmod(t, 4)
                    blocks = [(4 * kbp + j, qq * 256, j * 256, 256)
                              for j in range(4)]
                else:
                    blocks = [(t, ha * 512, ha * 512, 512) for ha in range(2)]
                for kb, qoff, coff, w in blocks:
                    nc.tensor.matmul(
                        qk[:, coff:coff + w],
                        lhsT=kT[mc][off:off + DK, kb * 128:(kb + 1) * 128],
                        rhs=qT[mc][off:off + DK,
                                   ih * IW + qoff: ih * IW + qoff + w],
                        start=True, stop=True)
                e = ep.tile([128, EXPW], bf16, tag="e", name=f"e{u}_{t}")
                nc.scalar.activation(out=e, in_=qk, func=AF.Exp,
                                     scale=1.0 / np.sqrt(DK))
                e_tiles[(u, t)] = e

            def e_slice(u, kb, qc):
                """lhsT slice [keys 128, queries 128] of unit u's e tiles."""
                if u == 0:
                    kbp = kb // 2
                    t = (qc // 4) * 4 + kbp if kbp < 4 \
                        else 8 + (qc // 4) * 4 + (kbp - 4)
                    off = (kb % 2) * 512 + (qc % 4) * 128
                elif u == SPLIT_U:
                    t = (qc // 2) * 4 + kb // 4
                    off = (kb % 4) * 256 + (qc % 2) * 128
                else:
                    t, off = kb, qc * 128
                return e_tiles[(u, t)][:, off:off + 128]

            pv_psum = {}

            def pv_qc(u, qc):
                """ctx[q 128, dk+1] for queries qc of unit u; accumulate all kb.
                Then normalize into stage (per-partition scalar multiply)."""
                ih, h = divmod(u, HPC)
                qg, q4 = divmod(qc, 4)
                if q4 == 0:
                    pv_psum[(u, qg)] = pvp.tile([128, 4, DK + 1], f32,
                                                tag="pv", name="pv")
                pv = pv_psum[(u, qg)]
                for kb in range(NKB):
                    nc.tensor.matmul(
                        pv[:, q4, :],
                        lhsT=e_slice(u, kb, qc),
                        rhs=v_sb[:, kb, h, :],
                        start=(kb == 0), stop=(kb == NKB - 1))
                inv = invp.tile([128, 1], f32, tag="inv", name="inv")
                nc.vector.reciprocal(out=inv, in_=pv[:, q4, DK:DK + 1])
                nc.vector.tensor_scalar_mul(
                    out=stage[ih][qc][:, h * DK:(h + 1) * DK],
                    in0=pv[:, q4, 0:DK], scalar1=inv)

            def t_qc(ih, qc, mc):
                """Transpose normalized ctx block [q 128, m 128] -> ctx_t."""
                ps = psp.tile([128, 512], bf16, tag="ps", name="tp")
                nc.tensor.transpose(
                    ps[:, 0:128],
                    in_=stage[ih][qc][:, mc * 128:(mc + 1) * 128],
                    identity=ident)
                nc.vector.tensor_copy(
                    out=ctx_t[ih][mc][:, qc * 128:(qc + 1) * 128],
                    in_=ps[:, 0:128])

            def wo_qc(ih, qc):
                """Output projection for query chunk qc of half ih."""
                for nh in range(2):
                    ps = psp.tile([128, 512], f32, tag="ps", name="wops")
                    for mc in range(NMC):
                        nc.tensor.matmul(
                            ps,
                            lhsT=ctx_t[ih][mc][:, qc * 128:(qc + 1) * 128],
                            rhs=wo_sb[:, mc, nh * 512:(nh + 1) * 512],
                            start=(mc == 0), stop=(mc == NMC - 1))
                    os_ = ostp.tile([128, 512], f32, tag="ost", name="ost")
                    nc.vector.tensor_copy(out=os_, in_=ps)
                    nc.sync.dma_start(
                        out=out_d[(ih * 8 + qc) * 128:(ih * 8 + qc + 1) * 128,
                                  nh * 512:(nh + 1) * 512],
                        in_=os_)

            # ---- static filler schedule ----
            PK = lambda tens, sh, mc, sc: (lambda: proj_mk(tens, sh, mc, sc))
            PJV = lambda kb: (lambda: proj_v(kb))
            PV = lambda u, qc: (lambda: pv_qc(u, qc))
            T = lambda ih, mc: (lambda: [t_qc(ih, qc, mc) for qc in range(8)])
            T1q = lambda qc: (lambda: t_qc(1, qc, 1))
            WO = lambda ih, qc: (lambda: wo_qc(ih, qc))

            FILL = {
                (0, 0): [PK("k", 0, 0, 1)],
                (0, 1): [PK("q", 0, 0, 1)],
                (0, 3): [PK("k", 1, 0, 0)],
                (0, 5): [PK("k", 1, 0, 1)],
                (0, 7): [PK("k", 0, 1, 0)],
                (0, 9): [PK("k", 0, 1, 1)],
                (0, 11): [PK("q", 0, 1, 0)],
                (0, 13): [PK("q", 0, 1, 1)],
                (0, 15): [PK("k", 1, 1, 0)],
                (1, 0): [PJV(0)], (1, 2): [PJV(1)], (1, 3): [PJV(2)],
                (1, 5): [PJV(3)], (1, 6): [PJV(4)], (1, 8): [PJV(5)],
                (1, 9): [PJV(6)], (1, 11): [PJV(7)], (1, 12): [PJV(8)],
                (1, 14): [PJV(9)], (1, 15): [PJV(10)],
                (2, 0): [PJV(11)], (2, 1): [PK("k", 1, 1, 1)],
                (2, 2): [PJV(12)], (2, 3): [PJV(13)], (2, 4): [PJV(14)],
                (2, 5): [PJV(15)],
                (2, 6): [PV(0, 0)], (2, 7): [PV(0, 1)], (2, 8): [PV(0, 2)],
                (2, 9): [PV(0, 3)], (2, 10): [PV(0, 4)], (2, 11): [PV(0, 5)],
                (2, 12): [PV(0, 6)], (2, 13): [PV(0, 7)],
                (3, 0): [PV(1, 0)], (3, 1): [PV(1, 1)], (3, 2): [PV(1, 2)],
                (3, 3): [PV(1, 3)], (3, 4): [PV(1, 4)], (3, 5): [PV(1, 5)],
                (3, 6): [PV(1, 6)], (3, 7): [PV(1, 7)],
                (3, 8): [PK("q", 1, 0, 0)], (3, 10): [PK("q", 1, 0, 1)],
                (3, 12): [PK("q", 1, 1, 0)], (3, 14): [PK("q", 1, 1, 1)],
                (4, 0): [PV(2, 0)], (4, 1): [PV(2, 1)], (4, 2): [PV(2, 2)],
                (4, 3): [PV(2, 3)], (4, 4): [PV(2, 4)], (4, 5): [PV(2, 5)],
                (4, 6): [PV(2, 6)], (4, 7): [PV(2, 7)],
                (4, 8): [PV(3, 0)], (4, 9): [PV(3, 1)], (4, 10): [PV(3, 2)],
                (4, 11): [PV(3, 3)], (4, 12): [PV(3, 4)], (4, 13): [PV(3, 5)],
                (4, 14): [PV(3, 6)], (4, 15): [PV(3, 7)],
                (5, 0): [T(0, 0)], (5, 1): [T(0, 1)],
                (5, 2): [WO(0, 0)], (5, 4): [WO(0, 1)], (5, 6): [WO(0, 2)],
                (5, 8): [WO(0, 3)], (5, 10): [WO(0, 4)], (5, 12): [WO(0, 5)],
                (5, 14): [WO(0, 6)], (5, 15): [WO(0, 7)],
                (6, 0): [PV(4, 0)], (6, 1): [PV(4, 1)], (6, 2): [PV(4, 2)],
                (6, 3): [PV(4, 3)], (6, 4): [PV(4, 4)], (6, 5): [PV(4, 5)],
                (6, 6): [PV(4, 6)], (6, 7): [PV(4, 7)],
                (6, 8): [PV(5, 0)], (6, 9): [PV(5, 1)], (6, 10): [PV(5, 2)],
                (6, 11): [PV(5, 3)], (6, 12): [PV(5, 4)], (6, 13): [PV(5, 5)],
                (6, 14): [PV(5, 6)], (6, 15): [PV(5, 7)],
                (7, 0): [T(1, 0)],
                (7, 1): [PV(6, 0)], (7, 2): [PV(6, 1)], (7, 3): [PV(6, 2)],
                (7, 4): [PV(6, 3)],
                (7, 5): [PV(7, 0)],
                (7, 6): [PV(7, 1), T1q(0)],
                (7, 7): [PV(6, 4), WO(1, 0)],
                (7, 8): [PV(6, 5), T1q(1)],
                (7, 9): [PV(7, 2), WO(1, 1)],
                (7, 10): [PV(7, 3), PV(6, 6)],
                (7, 11): [PV(6, 7), T1q(2), T1q(3)],
                (7, 12): [PV(7, 4), WO(1, 2)],
                (7, 13): [PV(7, 5), WO(1, 3), T1q(4)],
                (7, 14): [T1q(5), WO(1, 4)],
                (7, 15): [WO(1, 5)],
            }

            def proj_mk_pair(tens, sh, mc):
                """Both 512-col groups of a projection, dc-major interleaved
                so the last matmuls land right as the final x chunk arrives."""
                w_sb = wq_sb if tens == "q" else wk_sb
                pss = [psp.tile([128, 512], f32, tag="ps", name="ps")
                       for _ in range(2)]
                for dc in range(NDC):
                    for sc in range(2):
                        nc.tensor.matmul(
                            pss[sc],
                            lhsT=w_sb[:, dc, mc * 128:(mc + 1) * 128],
                            rhs=xc[(tens, sh, sc)][:, dc, :],
                            start=(dc == 0), stop=(dc == NDC - 1))
                for sc in range(2):
                    dst = (qT if tens == "q" else kT)[mc][
                        :, sh * 1024 + sc * 512: sh * 1024 + (sc + 1) * 512]
                    if tens == "q":
                        nc.vector.tensor_scalar_add(
                            out=dst, in0=pss[sc], scalar1=bq_sb[:, mc:mc + 1])
                    else:
                        nc.vector.tensor_copy(out=dst, in_=pss[sc])

            # ---- main pipeline ----
            # prologue: first projections (DMA-gated)
            proj_mk("k", 0, 0, 0)
            proj_mk("q", 0, 0, 0)
            # 8 attention units paced by the exp stream
            for u in range(8):
                for t in range(NT):
                    qk_tile(u, t)
                    for fn in FILL.get((u, t), []):
                        fn()
            # tail: close out the last two query-chunks
            pv_qc(7, 6)
            pv_qc(7, 7)
            t_qc(1, 6, 1)
            wo_qc(1, 6)
            t_qc(1, 7, 1)
            wo_qc(1, 7)

    nc.compile()
    return nc


def _get_nc(debug=False):
    key = ("nc", debug)
    if key not in _cached:
        _cached[key] = _build(debug)
    return _cached[key]


def _get_runner():
    """Build (once) a jitted 8-core SPMD executable mirroring
    bass2jax.run_bass_via_pjrt, reusable across calls for benchmarking."""
    if "runner" in _cached:
        return _cached["runner"]
    import jax
    import jax.numpy as jnp
    from jax.experimental.shard_map import shard_map
    from jax.sharding import Mesh, PartitionSpec
    import concourse.mybir as mybir
    from concourse import bass2jax

    bass2jax.install_neuronx_cc_hook()
    nc = _get_nc()
    assert nc.dbg_addr is None
    partition_name = nc.partition_id_tensor.name if nc.partition_id_tensor else None

    in_names, out_names, out_avals, zero_outs = [], [], [], []
    for alloc in nc.m.functions[0].allocations:
        if not isinstance(alloc, mybir.MemoryLocationSet):
            continue
        name = alloc.memorylocations[0].name
        if alloc.kind == "ExternalInput":
            if name != partition_name:
                in_names.append(name)
        elif alloc.kind == "ExternalOutput":
            out_names.append(name)
            shape = tuple(alloc.tensor_shape)
            dtype = mybir.dt.np(alloc.dtype)
            out_avals.append(jax.core.ShapedArray(shape, dtype))
            zero_outs.append(np.zeros(shape, dtype))
    n_params = len(in_names)
    all_in_names = in_names + out_names
    if partition_name is not None:
        all_in_names = all_in_names + [partition_name]
    donate = tuple(range(n_params, n_params + len(out_names)))

    def _body(*args):
        operands = list(args)
        if partition_name is not None:
            operands.append(bass2jax.partition_id_tensor())
        outs = bass2jax._bass_exec_p.bind(
            *operands,
            out_avals=tuple(out_avals),
            in_names=tuple(all_in_names),
            out_names=tuple(out_names),
            lowering_input_output_aliases=(),
            sim_require_finite=True,
            sim_require_nnan=True,
            nc=nc,
        )
        return tuple(outs)

    devices = jax.devices()[:NC]
    mesh = Mesh(np.asarray(devices), ("core",))
    nin = n_params + len(out_names)
    sharded = jax.jit(
        shard_map(
            _body,
            mesh=mesh,
            in_specs=(PartitionSpec("core"),) * nin,
            out_specs=(PartitionSpec("core"),) * len(out_names),
            check_rep=False,
        ),
        donate_argnums=donate,
        keep_unused=True,
    )

    def run(in_maps):
        concat_in = [
            np.concatenate([np.asarray(in_maps[c][n]) for c in range(NC)], axis=0)
            for n in in_names
        ]
        concat_zeros = [
            np.zeros((NC * z.shape[0], *z.shape[1:]), z.dtype) for z in zero_outs
        ]
        out_arrs = sharded(*concat_in, *concat_zeros)
        return [
            {
                n: np.asarray(out_arrs[i]).reshape(NC, *out_avals[i].shape)[c]
                for i, n in enumerate(out_names)
            }
            for c in range(NC)
        ]

    _cached["runner"] = (run, sharded, in_names, out_names, out_avals, zero_outs)
    return _cached["runner"]


def _make_in_maps(query, key, value, Wq, bq, Wk, bk, Wv, bv, Wo, bo):
    import ml_dtypes
    bf16 = ml_dtypes.bfloat16

    query = np.asarray(query, dtype=np.float32)
    key = np.asarray(key, dtype=np.float32)
    value = np.asarray(value, dtype=np.float32)
    Wq, Wk, Wv, Wo = (np.asarray(a, dtype=np.float32) for a in (Wq, Wk, Wv, Wo))
    bq = np.asarray(bq, dtype=np.float32)
    B = query.shape[0]
    ident = np.eye(128, dtype=bf16)

    xqT = [np.ascontiguousarray(query[b].T).astype(bf16) for b in range(B)]
    xkT = [np.ascontiguousarray(key[b].T).astype(bf16) for b in range(B)]
    xvT = [np.ascontiguousarray(value[b].T).astype(bf16) for b in range(B)]

    in_maps = []
    for c in range(NC):
        b, hg = divmod(c, NC // B)
        sl = slice(hg * M, (hg + 1) * M)
        in_maps.append(
            {
                "xqT": xqT[b],
                "xkT": xkT[b],
                "xvT": xvT[b],
                "wq": np.ascontiguousarray(Wq[:, sl]).astype(bf16),
                "wk": np.ascontiguousarray(Wk[:, sl]).astype(bf16),
                "wv": np.ascontiguousarray(Wv[:, sl]).astype(bf16),
                "wo": np.ascontiguousarray(Wo[sl, :]).astype(bf16),
                "bq": np.ascontiguousarray(bq[sl]),
                "ident": ident,
            }
        )
    return in_maps


def kernel(query, key, value, Wq, bq, Wk, bk, Wv, bv, Wo, bo):
    in_maps = _make_in_maps(query, key, value, Wq, bq, Wk, bk, Wv, bv, Wo, bo)
    run = _get_runner()[0]
    results = run(in_maps)

    B = np.asarray(query).shape[0]
    bo = np.asarray(bo, dtype=np.float32)
    bv = np.asarray(bv, dtype=np.float32)
    Wo_f = np.asarray(Wo, dtype=np.float32)
    base = bo + bv @ Wo_f  # bv contributes exactly bv @ Wo (sum of attn = 1)
    full = np.zeros((B, S, D), np.float32)
    for b in range(B):
        acc = np.zeros((S, D), np.float32)
        for g in range(NC // B):
            acc += results[b * (NC // B) + g]["out"]
        full[b] = acc + base[None, :]
    return full


# revision 50
# speedup vs baseline: 1.0022x; 1.0022x over previous
"""Multi-head attention (B=2, S=2048, D=1024, H=16) on 8 TRN2 NeuronCores.

Sharding: (batch, head-group) — core c handles batch c//4 and heads
[4*(c%4), 4*(c%4)+4). Each core projects its batch's tokens onto its 4 heads'
column-shards of Wq/Wk/Wv, runs attention for those heads, and multiplies by
its row-shard of Wo, producing a partial [S, D] output. The host sums the 4
partials per batch and adds (bo + bv @ Wo). bk is dropped entirely (a key
bias shifts every score of a query by the same constant, which softmax
cancels); bv contributes exactly bv @ Wo because attention weights sum to 1.

Device design notes:
  - All matmuls run in bf16 (1 cycle/row at any free size in the PE cost
    model); inputs and weights are cast to bf16 on the host, halving input
    DMA (22 MB -> 14 MB per core). PSUM accumulation stays f32.
  - Q/K project feature-major (out [m, s]; W stationary). V projects
    token-major (x stationary, Wv moving), giving v in [keys, dk] layout
    directly — no V transpose pass. A constant-1 column appended to each
    per-(key-chunk, head) V block produces the softmax denominator inside
    the PV matmul.
  - Scores are computed transposed (S^T [key, query]) in [128, 1024] PSUM
    tiles (2 banks, double buffered) feeding one Exp per tile on the Act
    engine — Act is the #2 engine at ~133 us busy and must never starve.
  - PV computes ctx with queries on the OUTPUT partitions: out[q, dk+1] +=
    e-slice^T @ v-chunk. Free dim is 65 instead of 512, so PV costs half
    the PE cycles of the untransposed form, and softmax normalization
    becomes a per-partition scalar op (DVE reciprocal + tensor_scalar_mul).
    Each query-chunk's 16-step PSUM accumulation is one contiguous matmul
    group — interleaving open accumulation groups in a bank miscomputes.
  - ctx [q, m] is then PE-transposed per 128x128 block into ctx^T [m, q]
    for the Wo projection (contraction over m).
  - Emission order IS each engine's execution order. The schedule runs 8
    attention units (query-half x head) paced by the exp stream, with a
    static filler table interleaving projections, PV of earlier units,
    transposes and Wo chunks between score tiles. x streams in as
    [128, 8 dc, 512] column-chunk DMAs (few DMAs; HWDGE costs ~625ns each,
    serialized). Unit 0 splits its score tiles by query-half so exp starts
    before the second half of xq arrives; unit 7 splits by query-quarter
    so most of its close-out (PV/transpose/Wo) overlaps its own exp tail.
    A 30-matmul zero warmup ramps the PE clock out of its cold p-state
    before the first DMA-gated projections.
  - PSUM: 2x1-bank pool (projections/transposes/Wo), 2x2-bank score tiles,
    2x1-bank PV accumulators ([128, 4, 65] f32). Total exactly 8 banks.

Measured (8-core run vs fp32 reference): rel err 5.4e-3 (gate 2e-2).
TimelineSim cost-model estimate: ~176.4 us per core (baseline was 254.6).
"""

import numpy as np

S = 2048          # sequence length
D = 1024          # model dim
HPC = 4           # heads per core
DK = 64           # head dim
M = HPC * DK      # per-core projection width = 256
NC = 8            # cores
IW = 1024         # attention query width per ih-half
NDC = D // 128    # 8 contraction chunks
NMC = M // 128    # 2 m-chunks (head pairs)
NKB = S // 128    # 16 key chunks
EXPW = 1024       # exp tile width
NT = 16 * IW // EXPW  # qk/exp tiles per unit = 16

_cached = {}


def _build(debug=False):
    import concourse.bass as bass
    import concourse.bacc as bacc
    import concourse.tile as tile
    import concourse.mybir as mybir
    from contextlib import ExitStack

    f32 = mybir.dt.float32
    bf16 = mybir.dt.bfloat16
    AF = mybir.ActivationFunctionType

    nc = bacc.Bacc(
        "TRN2",
        target_bir_lowering=False,
        debug=False,
        enable_asserts=False,
        num_devices=NC,
    )

    # DRAM I/O (per-core shapes)
    xqT_d = nc.dram_tensor("xqT", [D, S], bf16, kind="ExternalInput").ap()
    xkT_d = nc.dram_tensor("xkT", [D, S], bf16, kind="ExternalInput").ap()
    xvT_d = nc.dram_tensor("xvT", [D, S], bf16, kind="ExternalInput").ap()
    wq_d = nc.dram_tensor("wq", [D, M], bf16, kind="ExternalInput").ap()
    wk_d = nc.dram_tensor("wk", [D, M], bf16, kind="ExternalInput").ap()
    wv_d = nc.dram_tensor("wv", [D, M], bf16, kind="ExternalInput").ap()
    wo_d = nc.dram_tensor("wo", [M, D], bf16, kind="ExternalInput").ap()
    bq_d = nc.dram_tensor("bq", [M], f32, kind="ExternalInput").ap()
    ident_d = nc.dram_tensor("ident", [128, 128], bf16, kind="ExternalInput").ap()
    out_d = nc.dram_tensor("out", [S, D], f32, kind="ExternalOutput").ap()

    with tile.TileContext(nc) as tc:
        with ExitStack() as st:
            wp = st.enter_context(tc.tile_pool(name="wp", bufs=1))
            xt = st.enter_context(tc.tile_pool(name="xt", bufs=6))
            qkt = st.enter_context(tc.tile_pool(name="qkt", bufs=1))
            vp = st.enter_context(tc.tile_pool(name="vp", bufs=1))
            ep = st.enter_context(tc.tile_pool(name="ep", bufs=46))
            stp = st.enter_context(tc.tile_pool(name="stp", bufs=1))
            ctp = st.enter_context(tc.tile_pool(name="ctp", bufs=1))
            invp = st.enter_context(tc.tile_pool(name="invp", bufs=4))
            ostp = st.enter_context(tc.tile_pool(name="ostp", bufs=4))
            # PSUM: exactly 8 banks
            psp = st.enter_context(tc.tile_pool(name="psp", bufs=2, space="PSUM"))
            qkp = st.enter_context(tc.tile_pool(name="qkp", bufs=2, space="PSUM"))
            pvp = st.enter_context(tc.tile_pool(name="pvp", bufs=2, space="PSUM"))

            wq_sb = wp.tile([128, NDC, M], bf16, tag="wq")
            wk_sb = wp.tile([128, NDC, M], bf16, tag="wk")
            wv_sb = wp.tile([128, NDC, M], bf16, tag="wv")
            wo_sb = wp.tile([128, NMC, D], bf16, tag="wo")
            bq_sb = wp.tile([128, NMC], f32, tag="bq")
            ident = wp.tile([128, 128], bf16, tag="ident")
            qT = [qkt.tile([128, S], bf16, tag=f"qT{m}", name=f"qT{m}")
                  for m in range(NMC)]
            kT = [qkt.tile([128, S], bf16, tag=f"kT{m}", name=f"kT{m}")
                  for m in range(NMC)]
            # v in [keys, head, dk+1] layout; col DK is the constant 1
            v_sb = vp.tile([128, NKB, HPC, DK + 1], bf16, tag="v")
            stage = [[stp.tile([128, M], bf16, tag=f"st{i}{q}", name=f"st{i}{q}")
                      for q in range(8)] for i in range(2)]
            ctx_t = [[ctp.tile([128, IW], bf16, tag=f"ct{i}{m}", name=f"ct{i}{m}")
                      for m in range(NMC)] for i in range(2)]

            nc.vector.memset(v_sb[:, :, :, DK:DK + 1], 1.0)
            warm = wp.tile([128, 512], bf16, tag="warm")
            nc.vector.memset(warm, 0.0)
            wps = psp.tile([128, 512], f32, tag="ps", name="warmps")
            for _ in range(30):
                nc.tensor.matmul(wps, lhsT=warm[:, 0:128], rhs=warm,
                                 start=True, stop=True)

            w_r = lambda ap: ap.rearrange("(n p) m -> p n m", p=128)

            # ---- input DMA emission (order = SP.SEQ issue order) ----
            xc = {}   # (tensor, sh, sc) -> [128, 8, 512] tile

            def load_x(tens, dram, sh, sc):
                t = xt.tile([128, NDC, 512], bf16, tag="x",
                            name=f"x{tens}{sh}{sc}")
                nc.sync.dma_start(
                    out=t,
                    in_=dram[:, sh * 1024 + sc * 512:
                             sh * 1024 + (sc + 1) * 512].rearrange(
                                 "(n p) s -> p n s", p=128))
                xc[(tens, sh, sc)] = t

            nc.sync.dma_start(out=wk_sb, in_=w_r(wk_d))
            nc.sync.dma_start(out=wq_sb, in_=w_r(wq_d))
            load_x("k", xkT_d, 0, 0)
            load_x("q", xqT_d, 0, 0)
            nc.sync.dma_start(out=bq_sb, in_=bq_d.rearrange("(n p) -> p n", p=128))
            load_x("k", xkT_d, 0, 1)
            load_x("q", xqT_d, 0, 1)
            load_x("k", xkT_d, 1, 0)
            load_x("k", xkT_d, 1, 1)
            nc.sync.dma_start(out=wv_sb, in_=w_r(wv_d))
            load_x("v", xvT_d, 0, 0)
            load_x("v", xvT_d, 0, 1)
            load_x("v", xvT_d, 1, 0)
            load_x("v", xvT_d, 1, 1)
            load_x("q", xqT_d, 1, 0)
            load_x("q", xqT_d, 1, 1)
            nc.sync.dma_start(out=wo_sb, in_=wo_d.rearrange("(g p) n -> p g n", p=128))
            nc.sync.dma_start(out=ident, in_=ident_d)

            # ---- emission helpers ----
            def proj_mk(tens, sh, mc, sc):
                """Project q/k chunk: out [m 128, s 512]; W stationary."""
                w_sb = wq_sb if tens == "q" else wk_sb
                ps = psp.tile([128, 512], f32, tag="ps", name="ps")
                for dc in range(NDC):
                    nc.tensor.matmul(
                        ps,
                        lhsT=w_sb[:, dc, mc * 128:(mc + 1) * 128],
                        rhs=xc[(tens, sh, sc)][:, dc, :],
                        start=(dc == 0), stop=(dc == NDC - 1))
                dst = (qT if tens == "q" else kT)[mc][
                    :, sh * 1024 + sc * 512: sh * 1024 + (sc + 1) * 512]
                if tens == "q":
                    nc.vector.tensor_scalar_add(
                        out=dst, in0=ps, scalar1=bq_sb[:, mc:mc + 1])
                else:
                    nc.vector.tensor_copy(out=dst, in_=ps)

            def proj_v(kb):
                """Project v key-chunk kb: out [s 128, m 256]; x stationary."""
                sh, r = divmod(kb, 8)
                sc, q = divmod(r, 4)
                ps = psp.tile([128, 512], f32, tag="ps", name="ps")
                for dc in range(NDC):
                    nc.tensor.matmul(
                        ps[:, 0:M],
                        lhsT=xc[("v", sh, sc)][:, dc, q * 128:(q + 1) * 128],
                        rhs=wv_sb[:, dc, :],
                        start=(dc == 0), stop=(dc == NDC - 1))
                for h in range(HPC):
                    nc.vector.tensor_copy(
                        out=v_sb[:, kb, h, 0:DK],
                        in_=ps[:, h * DK:(h + 1) * DK])

            e_tiles = {}
            SPLIT_U = 7  # last unit: tiles cover [4 kb x 256 queries] so each
                         # query-quarter closes out while later quarters exp

            def qk_tile(u, t):
                """Scores^T tile [keys 128, 1024] + exp -> e (bf16)."""
                ih, h = divmod(u, HPC)
                mc, off = divmod(h, 2)
                off *= DK
                qk = qkp.tile([128, EXPW], f32, tag="qk", name="qk")
                if u == 0:
                    base, r = (0, t) if t < 8 else (4, t - 8)
                    qh, kbp = divmod(r, 4)
                    blocks = [(2 * (base + kbp) + j, qh * 512, j * 512, 512)
                              for j in range(2)]
                elif u == SPLIT_U:
                    qq, kbp = divmod(t, 4)
                    blocks = [(4 * kbp + j, qq * 256, j * 256, 256)
                              for j in range(4)]
                else:
                    blocks = [(t, ha * 512, ha * 512, 512) for ha in range(2)]
                for kb, qoff, coff, w in blocks:
                    nc.tensor.matmul(
                        qk[:, coff:coff + w],
                        lhsT=kT[mc][off:off + DK, kb * 128:(kb + 1) * 128],
                        rhs=qT[mc][off:off + DK,
                                   ih * IW + qoff: ih * IW + qoff + w],
                        start=True, stop=True)
                e = ep.tile([128, EXPW], bf16, tag="e", name=f"e{u}_{t}")
                nc.scalar.activation(out=e, in_=qk, func=AF.Exp,
                                     scale=1.0 / np.sqrt(DK))
                e_tiles[(u, t)] = e

            def e_slice(u, kb, qc):
                """lhsT slice [keys 128, queries 128] of unit u's e tiles."""
                if u == 0:
                    kbp = kb // 2
                    t = (qc // 4) * 4 + kbp if kbp < 4 \
                        else 8 + (qc // 4) * 4 + (kbp - 4)
                    off = (kb % 2) * 512 + (qc % 4) * 128
                elif u == SPLIT_U:
                    t = (qc // 2) * 4 + kb // 4
                    off = (kb % 4) * 256 + (qc % 2) * 128
                else:
                    t, off = kb, qc * 128
                return e_tiles[(u, t)][:, off:off + 128]

            pv_psum = {}

            def pv_qc(u, qc):
                """ctx[q 128, dk+1] for queries qc of unit u; accumulate all kb.
                Then normalize into stage (per-partition scalar multiply)."""
                ih, h = divmod(u, HPC)
                qg, q4 = divmod(qc, 4)
                if q4 == 0:
                    pv_psum[(u, qg)] = pvp.tile([128, 4, DK + 1], f32,
                                                tag="pv", name="pv")
                pv = pv_psum[(u, qg)]
                for kb in range(NKB):
                    nc.tensor.matmul(
                        pv[:, q4, :],
                        lhsT=e_slice(u, kb, qc),
                        rhs=v_sb[:, kb, h, :],
                        start=(kb == 0), stop=(kb == NKB - 1))
                inv = invp.tile([128, 1], f32, tag="inv", name="inv")
                nc.vector.reciprocal(out=inv, in_=pv[:, q4, DK:DK + 1])
                nc.vector.tensor_scalar_mul(
                    out=stage[ih][qc][:, h * DK:(h + 1) * DK],
                    in0=pv[:, q4, 0:DK], scalar1=inv)

            def t_qc(ih, qc, mc):
                """Transpose normalized ctx block [q 128, m 128] -> ctx_t."""
                ps = psp.tile([128, 512], bf16, tag="ps", name="tp")
                nc.tensor.transpose(
                    ps[:, 0:128],
                    in_=stage[ih][qc][:, mc * 128:(mc + 1) * 128],
                    identity=ident)
                nc.vector.tensor_copy(
                    out=ctx_t[ih][mc][:, qc * 128:(qc + 1) * 128],
                    in_=ps[:, 0:128])

            def wo_qc(ih, qc):
                """Output projection for query chunk qc of half ih."""
                for nh in range(2):
                    ps = psp.tile([128, 512], f32, tag="ps", name="wops")
                    for mc in range(NMC):
                        nc.tensor.matmul(
                            ps,
                            lhsT=ctx_t[ih][mc][:, qc * 128:(qc + 1) * 128],
                            rhs=wo_sb[:, mc, nh * 512:(nh + 1) * 512],
                            start=(mc == 0), stop=(mc == NMC - 1))
                    os_ = ostp.tile([128, 512], f32, tag="ost", name="ost")
                    nc.vector.tensor_copy(out=os_, in_=ps)
                    nc.sync.dma_start(
                        out=out_d[(ih * 8 + qc) * 128:(ih * 8 + qc + 1) * 128,
                                  nh * 512:(nh + 1) * 512],
                        in_=os_)

            # ---- static filler schedule ----
            PK = lambda tens, sh, mc, sc: (lambda: proj_mk(tens, sh, mc, sc))
            PJV = lambda kb: (lambda: proj_v(kb))
            PV = lambda u, qc: (lambda: pv_qc(u, qc))
            T = lambda ih, mc: (lambda: [t_qc(ih, qc, mc) for qc in range(8)])
            T1q = lambda qc: (lambda: t_qc(1, qc, 1))
            WO = lambda ih, qc: (lambda: wo_qc(ih, qc))

            FILL = {
                (0, 0): [PK("k", 0, 0, 1)],
                (0, 1): [PK("q", 0, 0, 1)],
                (0, 3): [PK("k", 1, 0, 0)],
                (0, 5): [PK("k", 1, 0, 1)],
                (0, 7): [PK("k", 0, 1, 0)],
                (0, 9): [PK("k", 0, 1, 1)],
                (0, 11): [PK("q", 0, 1, 0)],
                (0, 13): [PK("q", 0, 1, 1)],
                (0, 15): [PK("k", 1, 1, 0)],
                (1, 0): [PJV(0)], (1, 2): [PJV(1)], (1, 3): [PJV(2)],
                (1, 5): [PJV(3)], (1, 6): [PJV(4)], (1, 8): [PJV(5)],
                (1, 9): [PJV(6)], (1, 11): [PJV(7)], (1, 12): [PJV(8)],
                (1, 14): [PJV(9)], (1, 15): [PJV(10)],
                (2, 0): [PJV(11)], (2, 1): [PK("k", 1, 1, 1)],
                (2, 2): [PJV(12)], (2, 3): [PJV(13)], (2, 4): [PJV(14)],
                (2, 5): [PJV(15)],
                (2, 6): [PV(0, 0)], (2, 7): [PV(0, 1)], (2, 8): [PV(0, 2)],
                (2, 9): [PV(0, 3)], (2, 10): [PV(0, 4)], (2, 11): [PV(0, 5)],
                (2, 12): [PV(0, 6)], (2, 13): [PV(0, 7)],
                (3, 0): [PV(1, 0)], (3, 1): [PV(1, 1)], (3, 2): [PV(1, 2)],
                (3, 3): [PV(1, 3)], (3, 4): [PV(1, 4)], (3, 5): [PV(1, 5)],
                (3, 6): [PV(1, 6)], (3, 7): [PV(1, 7)],
                (3, 8): [PK("q", 1, 0, 0)], (3, 10): [PK("q", 1, 0, 1)],
                (3, 12): [PK("q", 1, 1, 0)], (3, 14): [PK("q", 1, 1, 1)],
                (4, 0): [PV(2, 0)], (4, 1): [PV(2, 1)], (4, 2): [PV(2, 2)],
                (4, 3): [PV(2, 3)], (4, 4): [PV(2, 4)], (4, 5): [PV(2, 5)],
                (4, 6): [PV(2, 6)], (4, 7): [PV(2, 7)],
                (4, 8): [PV(3, 0)], (4, 9): [PV(3, 1)], (4, 10): [PV(3, 2)],
                (4, 11): [PV(3, 3)], (4, 12): [PV(3, 4)], (4, 13): [PV(3, 5)],
                (4, 14): [PV(3, 6)], (4, 15): [PV(3, 7)],
                (5, 0): [T(0, 0)], (5, 1): [T(0, 1)],
                (5, 2): [WO(0, 0)], (5, 4): [WO(0, 1)], (5, 6): [WO(0, 2)],
                (5, 8): [WO(0, 3)], (5, 10): [WO(0, 4)], (5, 12): [WO(0, 5)],
                (5, 14): [WO(0, 6)], (5, 15): [WO(0, 7)],
                (6, 0): [PV(4, 0)], (6, 1): [PV(4, 1)], (6, 2): [PV(4, 2)],
                (6, 3): [PV(4, 3)], (6, 4): [PV(4, 4)], (6, 5): [PV(4, 5)],
                (6, 6): [PV(4, 6)], (6, 7): [PV(4, 7)],
                (6, 8): [PV(5, 0)], (6, 9): [PV(5, 1)], (6, 10): [PV(5, 2)],
                (6, 11): [PV(5, 3)], (6, 12): [PV(5, 4)], (6, 13): [PV(5, 5)],
                (6, 14): [PV(5, 6)], (6, 15): [PV(5, 7)],
                (7, 0): [T(1, 0)],
                (7, 1): [PV(6, 0)], (7, 2): [PV(6, 1)], (7, 3): [PV(6, 2)],
                (7, 4): [PV(6, 3)],
                (7, 5): [PV(7, 0)],
                (7, 6): [PV(7, 1), T1q(0)],
                (7, 7): [PV(6, 4), WO(1, 0)],
                (7, 8): [PV(6, 5), T1q(1)],
                (7, 9): [PV(7, 2), WO(1, 1)],
                (7, 10): [PV(7, 3), PV(6, 6)],
                (7, 11): [PV(6, 7), T1q(2), T1q(3)],
                (7, 12): [PV(7, 4), WO(1, 2)],
                (7, 13): [PV(7, 5), WO(1, 3), T1q(4)],
                (7, 14): [T1q(5), WO(1, 4)],
                (7, 15): [WO(1, 5)],
            }

            def proj_mk_pair(tens, sh, mc):
                """Both 512-col groups of a projection, dc-major interleaved
                so the last matmuls land right as the final x chunk arrives."""
                w_sb = wq_sb if tens == "q" else wk_sb
                pss = [psp.tile([128, 512], f32, tag="ps", name="ps")
                       for _ in range(2)]
                for dc in range(NDC):
                    for sc in range(2):
                        nc.tensor.matmul(
                            pss[sc],
                            lhsT=w_sb[:, dc, mc * 128:(mc + 1) * 128],
                            rhs=xc[(tens, sh, sc)][:, dc, :],
                            start=(dc == 0), stop=(dc == NDC - 1))
                for sc in range(2):
                    dst = (qT if tens == "q" else kT)[mc][
                        :, sh * 1024 + sc * 512: sh * 1024 + (sc + 1) * 512]
                    if tens == "q":
                        nc.vector.tensor_scalar_add(
                            out=dst, in0=pss[sc], scalar1=bq_sb[:, mc:mc + 1])
                    else:
                        nc.vector.tensor_copy(out=dst, in_=pss[sc])

            # ---- main pipeline ----
            # prologue: first projections (DMA-gated)
            proj_mk("k", 0, 0, 0)
            proj_mk("q", 0, 0, 0)
            # 8 attention units paced by the exp stream
            for u in range(8):
                for t in range(NT):
                    qk_tile(u, t)
                    for fn in FILL.get((u, t), []):
                        fn()
            # tail: close out the last two query-chunks
            pv_qc(7, 6)
            pv_qc(7, 7)
            t_qc(1, 6, 1)
            wo_qc(1, 6)
            t_qc(1, 7, 1)
            wo_qc(1, 7)

    nc.compile()
    return nc


def _get_nc(debug=False):
    key = ("nc", debug)
    if key not in _cached:
        _cached[key] = _build(debug)
    return _cached[key]


def _get_runner():
    """Build (once) a jitted 8-core SPMD executable mirroring
    bass2jax.run_bass_via_pjrt, reusable across calls for benchmarking."""
    if "runner" in _cached:
        return _cached["runner"]
    import jax
    import jax.numpy as jnp
    from jax.experimental.shard_map import shard_map
    from jax.sharding import Mesh, PartitionSpec
    import concourse.mybir as mybir
    from concourse import bass2jax

    bass2jax.install_neuronx_cc_hook()
    nc = _get_nc()
    assert nc.dbg_addr is None
    partition_name = nc.partition_id_tensor.name if nc.partition_id_tensor else None

    in_names, out_names, out_avals, zero_outs = [], [], [], []
    for alloc in nc.m.functions[0].allocations:
        if not isinstance(alloc, mybir.MemoryLocationSet):
            continue
        name = alloc.memorylocations[0].name
        if alloc.kind == "ExternalInput":
            if name != partition_name:
                in_names.append(name)
        elif alloc.kind == "ExternalOutput":
            out_names.append(name)
            shape = tuple(alloc.tensor_shape)
            dtype = mybir.dt.np(alloc.dtype)
            out_avals.append(jax.core.ShapedArray(shape, dtype))
            zero_outs.append(np.zeros(shape, dtype))
    n_params = len(in_names)
    all_in_names = in_names + out_names
    if partition_name is not None:
        all_in_names = all_in_names + [partition_name]
    donate = tuple(range(n_params, n_params + len(out_names)))

    def _body(*args):
        operands = list(args)
        if partition_name is not None:
            operands.append(bass2jax.partition_id_tensor())
        outs = bass2jax._bass_exec_p.bind(
            *operands,
            out_avals=tuple(out_avals),
            in_names=tuple(all_in_names),
            out_names=tuple(out_names),
            lowering_input_output_aliases=(),
            sim_require_finite=True,
            sim_require_nnan=True,
            nc=nc,
        )
        return tuple(outs)

    devices = jax.devices()[:NC]
    mesh = Mesh(np.asarray(devices), ("core",))
    nin = n_params + len(out_names)
    sharded = jax.jit(
        shard_map(
            _body,
            mesh=mesh,
            in_specs=(PartitionSpec("core"),) * nin,
            out_specs=(PartitionSpec("core"),) * len(out_names),
            check_rep=False,
        ),
        donate_argnums=donate,
        keep_unused=True,
    )

    def run(in_maps):
        concat_in = [
            np.concatenate([np.asarray(in_maps[c][n]) for c in range(NC)], axis=0)
            for n in in_names
        ]
        concat_zeros = [
            np.zeros((NC * z.shape[0], *z.shape[1:]), z.dtype) for z in zero_outs
        ]
        out_arrs = sharded(*concat_in, *concat_zeros)
        return [
            {
                n: np.asarray(out_arrs[i]).reshape(NC, *out_avals[i].shape)[c]
                for i, n in enumerate(out_names)
            }
            for c in range(NC)
        ]

    _cached["runner"] = (run, sharded, in_names, out_names, out_avals, zero_outs)
    return _cached["runner"]


def _make_in_maps(query, key, value, Wq, bq, Wk, bk, Wv, bv, Wo, bo):
    import ml_dtypes
    bf16 = ml_dtypes.bfloat16

    query = np.asarray(query, dtype=np.float32)
    key = np.asarray(key, dtype=np.float32)
    value = np.asarray(value, dtype=np.float32)
    Wq, Wk, Wv, Wo = (np.asarray(a, dtype=np.float32) for a in (Wq, Wk, Wv, Wo))
    bq = np.asarray(bq, dtype=np.float32)
    B = query.shape[0]
    ident = np.eye(128, dtype=bf16)

    xqT = [np.ascontiguousarray(query[b].T).astype(bf16) for b in range(B)]
    xkT = [np.ascontiguousarray(key[b].T).astype(bf16) for b in range(B)]
    xvT = [np.ascontiguousarray(value[b].T).astype(bf16) for b in range(B)]

    in_maps = []
    for c in range(NC):
        b, hg = divmod(c, NC // B)
        sl = slice(hg * M, (hg + 1) * M)
        in_maps.append(
            {
                "xqT": xqT[b],
                "xkT": xkT[b],
                "xvT": xvT[b],
                "wq": np.ascontiguousarray(Wq[:, sl]).astype(bf16),
                "wk": np.ascontiguousarray(Wk[:, sl]).astype(bf16),
                "wv": np.ascontiguousarray(Wv[:, sl]).astype(bf16),
                "wo": np.ascontiguousarray(Wo[sl, :]).astype(bf16),
                "bq": np.ascontiguousarray(bq[sl]),
                "ident": ident,
            }
        )
    return in_maps


def kernel(query, key, value, Wq, bq, Wk, bk, Wv, bv, Wo, bo):
    in_maps = _make_in_maps(query, key, value, Wq, bq, Wk, bk, Wv, bv, Wo, bo)
    run = _get_runner()[0]
    results = run(in_maps)

    B = np.asarray(query).shape[0]
    bo = np.asarray(bo, dtype=np.float32)
    bv = np.asarray(bv, dtype=np.float32)
    Wo_f = np.asarray(Wo, dtype=np.float32)
    base = bo + bv @ Wo_f  # bv contributes exactly bv @ Wo (sum of attn = 1)
    full = np.zeros((B, S, D), np.float32)
    for b in range(B):
        acc = np.zeros((S, D), np.float32)
        for g in range(NC // B):
            acc += results[b * (NC // B) + g]["out"]
        full[b] = acc + base[None, :]
    return full


# revision 51
# speedup vs baseline: 1.0163x; 1.0141x over previous
"""Multi-head attention (B=2, S=2048, D=1024, H=16) on 8 TRN2 NeuronCores.

Sharding: (batch, head-group) — core c handles batch c//4 and heads
[4*(c%4), 4*(c%4)+4). Each core projects its batch's tokens onto its 4 heads'
column-shards of Wq/Wk/Wv, runs attention for those heads, and multiplies by
its row-shard of Wo, producing a partial [S, D] output. The host sums the 4
partials per batch and adds (bo + bv @ Wo). bk is dropped entirely (a key
bias shifts every score of a query by the same constant, which softmax
cancels); bv contributes exactly bv @ Wo because attention weights sum to 1.

Device design notes:
  - All matmuls run in bf16 (1 cycle/row at any free size in the PE cost
    model); inputs and weights are cast to bf16 on the host, halving input
    DMA (22 MB -> 14 MB per core). PSUM accumulation stays f32.
  - Q/K project feature-major (out [m, s]; W stationary). V projects
    token-major (x stationary, Wv moving), giving v in [keys, dk] layout
    directly — no V transpose pass. A constant-1 column appended to each
    per-(key-chunk, head) V block produces the softmax denominator inside
    the PV matmul.
  - Scores are computed transposed (S^T [key, query]) in [128, 1024] PSUM
    tiles (2 banks, double buffered) feeding one Exp per tile on the Act
    engine — Act is the #2 engine at ~133 us busy and must never starve.
  - PV computes ctx with queries on the OUTPUT partitions: out[q, dk+1] +=
    e-slice^T @ v-chunk. Free dim is 65 instead of 512, so PV costs half
    the PE cycles of the untransposed form, and softmax normalization
    becomes a per-partition scalar op (DVE reciprocal + tensor_scalar_mul).
    Each query-chunk's 16-step PSUM accumulation is one contiguous matmul
    group — interleaving open accumulation groups in a bank miscomputes.
  - ctx [q, m] is then PE-transposed per 128x128 block into ctx^T [m, q]
    for the Wo projection (contraction over m).
  - Emission order IS each engine's execution order. The schedule runs 8
    attention units (query-half x head) paced by the exp stream, with a
    static filler table interleaving projections, PV of earlier units,
    transposes and Wo chunks between score tiles. x streams in as
    [128, 8 dc, 512] column-chunk DMAs (few DMAs; HWDGE costs ~625ns each,
    serialized). Unit 0 splits its score tiles by query-half so exp starts
    before the second half of xq arrives; unit 7 splits by query-quarter
    so most of its close-out (PV/transpose/Wo) overlaps its own exp tail.
    A 30-matmul zero warmup ramps the PE clock out of its cold p-state
    before the first DMA-gated projections.
  - PSUM: 2x1-bank pool (projections/transposes/Wo), 2x2-bank score tiles,
    2x1-bank PV accumulators ([128, 4, 65] f32). Total exactly 8 banks.

Measured (8-core run vs fp32 reference): rel err 5.4e-3 (gate 2e-2).
TimelineSim cost-model estimate: ~176.4 us per core (baseline was 254.6).
"""

import numpy as np

S = 2048          # sequence length
D = 1024          # model dim
HPC = 4           # heads per core
DK = 64           # head dim
M = HPC * DK      # per-core projection width = 256
NC = 8            # cores
IW = 1024         # attention query width per ih-half
NDC = D // 128    # 8 contraction chunks
NMC = M // 128    # 2 m-chunks (head pairs)
NKB = S // 128    # 16 key chunks
EXPW = 1024       # exp tile width
NT = 16 * IW // EXPW  # qk/exp tiles per unit = 16

_cached = {}


def _build(debug=False):
    import concourse.bass as bass
    import concourse.bacc as bacc
    import concourse.tile as tile
    import concourse.mybir as mybir
    from contextlib import ExitStack

    f32 = mybir.dt.float32
    bf16 = mybir.dt.bfloat16
    AF = mybir.ActivationFunctionType

    nc = bacc.Bacc(
        "TRN2",
        target_bir_lowering=False,
        debug=False,
        enable_asserts=False,
        num_devices=NC,
    )

    # DRAM I/O (per-core shapes)
    xqT_d = nc.dram_tensor("xqT", [D, S], bf16, kind="ExternalInput").ap()
    xkT_d = nc.dram_tensor("xkT", [D, S], bf16, kind="ExternalInput").ap()
    xvT_d = nc.dram_tensor("xvT", [D, S], bf16, kind="ExternalInput").ap()
    wq_d = nc.dram_tensor("wq", [D, M], bf16, kind="ExternalInput").ap()
    wk_d = nc.dram_tensor("wk", [D, M], bf16, kind="ExternalInput").ap()
    wv_d = nc.dram_tensor("wv", [D, M], bf16, kind="ExternalInput").ap()
    wo_d = nc.dram_tensor("wo", [M, D], bf16, kind="ExternalInput").ap()
    bq_d = nc.dram_tensor("bq", [M], f32, kind="ExternalInput").ap()
    ident_d = nc.dram_tensor("ident", [128, 128], bf16, kind="ExternalInput").ap()
    out_d = nc.dram_tensor("out", [S, D], f32, kind="ExternalOutput").ap()

    with tile.TileContext(nc) as tc:
        with ExitStack() as st:
            wp = st.enter_context(tc.tile_pool(name="wp", bufs=1))
            xt = st.enter_context(tc.tile_pool(name="xt", bufs=6))
            qkt = st.enter_context(tc.tile_pool(name="qkt", bufs=1))
            vp = st.enter_context(tc.tile_pool(name="vp", bufs=1))
            ep = st.enter_context(tc.tile_pool(name="ep", bufs=46))
            stp = st.enter_context(tc.tile_pool(name="stp", bufs=1))
            ctp = st.enter_context(tc.tile_pool(name="ctp", bufs=1))
            invp = st.enter_context(tc.tile_pool(name="invp", bufs=4))
            ostp = st.enter_context(tc.tile_pool(name="ostp", bufs=4))
            # PSUM: exactly 8 banks
            psp = st.enter_context(tc.tile_pool(name="psp", bufs=2, space="PSUM"))
            qkp = st.enter_context(tc.tile_pool(name="qkp", bufs=2, space="PSUM"))
            pvp = st.enter_context(tc.tile_pool(name="pvp", bufs=2, space="PSUM"))

            wq_sb = wp.tile([128, NDC, M], bf16, tag="wq")
            wk_sb = wp.tile([128, NDC, M], bf16, tag="wk")
            wv_sb = wp.tile([128, NDC, M], bf16, tag="wv")
            wo_sb = wp.tile([128, NMC, D], bf16, tag="wo")
            bq_sb = wp.tile([128, NMC], f32, tag="bq")
            ident = wp.tile([128, 128], bf16, tag="ident")
            qT = [qkt.tile([128, S], bf16, tag=f"qT{m}", name=f"qT{m}")
                  for m in range(NMC)]
            kT = [qkt.tile([128, S], bf16, tag=f"kT{m}", name=f"kT{m}")
                  for m in range(NMC)]
            # v in [keys, head, dk+1] layout; col DK is the constant 1
            v_sb = vp.tile([128, NKB, HPC, DK + 1], bf16, tag="v")
            stage = [[stp.tile([128, M], bf16, tag=f"st{i}{q}", name=f"st{i}{q}")
                      for q in range(8)] for i in range(2)]
            ctx_t = [[ctp.tile([128, IW], bf16, tag=f"ct{i}{m}", name=f"ct{i}{m}")
                      for m in range(NMC)] for i in range(2)]

            nc.vector.memset(v_sb[:, :, :, DK:DK + 1], 1.0)
            warm = wp.tile([128, 512], bf16, tag="warm")
            nc.vector.memset(warm, 0.0)
            wps = psp.tile([128, 512], f32, tag="ps", name="warmps")
            for _ in range(30):
                nc.tensor.matmul(wps, lhsT=warm[:, 0:128], rhs=warm,
                                 start=True, stop=True)

            w_r = lambda ap: ap.rearrange("(n p) m -> p n m", p=128)

            # ---- input DMA emission (order = SP.SEQ issue order) ----
            xc = {}   # (tensor, sh, sc) -> [128, 8, 512] tile

            def load_x(tens, dram, sh, sc):
                t = xt.tile([128, NDC, 512], bf16, tag="x",
                            name=f"x{tens}{sh}{sc}")
                nc.sync.dma_start(
                    out=t,
                    in_=dram[:, sh * 1024 + sc * 512:
                             sh * 1024 + (sc + 1) * 512].rearrange(
                                 "(n p) s -> p n s", p=128))
                xc[(tens, sh, sc)] = t

            nc.sync.dma_start(out=wk_sb, in_=w_r(wk_d))
            nc.sync.dma_start(out=wq_sb, in_=w_r(wq_d))
            load_x("k", xkT_d, 0, 0)
            load_x("q", xqT_d, 0, 0)
            nc.sync.dma_start(out=bq_sb, in_=bq_d.rearrange("(n p) -> p n", p=128))
            load_x("k", xkT_d, 0, 1)
            load_x("q", xqT_d, 0, 1)
            load_x("k", xkT_d, 1, 0)
            load_x("k", xkT_d, 1, 1)
            nc.sync.dma_start(out=wv_sb, in_=w_r(wv_d))
            load_x("v", xvT_d, 0, 0)
            load_x("v", xvT_d, 0, 1)
            load_x("v", xvT_d, 1, 0)
            load_x("v", xvT_d, 1, 1)
            load_x("q", xqT_d, 1, 0)
            load_x("q", xqT_d, 1, 1)
            nc.sync.dma_start(out=wo_sb, in_=wo_d.rearrange("(g p) n -> p g n", p=128))
            nc.sync.dma_start(out=ident, in_=ident_d)

            # ---- emission helpers ----
            def proj_mk(tens, sh, mc, sc):
                """Project q/k chunk: out [m 128, s 512]; W stationary."""
                w_sb = wq_sb if tens == "q" else wk_sb
                ps = psp.tile([128, 512], f32, tag="ps", name="ps")
                for dc in range(NDC):
                    nc.tensor.matmul(
                        ps,
                        lhsT=w_sb[:, dc, mc * 128:(mc + 1) * 128],
                        rhs=xc[(tens, sh, sc)][:, dc, :],
                        start=(dc == 0), stop=(dc == NDC - 1))
                dst = (qT if tens == "q" else kT)[mc][
                    :, sh * 1024 + sc * 512: sh * 1024 + (sc + 1) * 512]
                if tens == "q":
                    nc.vector.tensor_scalar_add(
                        out=dst, in0=ps, scalar1=bq_sb[:, mc:mc + 1])
                else:
                    nc.vector.tensor_copy(out=dst, in_=ps)

            def proj_v(kb):
                """Project v key-chunk kb: out [s 128, m 256]; x stationary."""
                sh, r = divmod(kb, 8)
                sc, q = divmod(r, 4)
                ps = psp.tile([128, 512], f32, tag="ps", name="ps")
                for dc in range(NDC):
                    nc.tensor.matmul(
                        ps[:, 0:M],
                        lhsT=xc[("v", sh, sc)][:, dc, q * 128:(q + 1) * 128],
                        rhs=wv_sb[:, dc, :],
                        start=(dc == 0), stop=(dc == NDC - 1))
                for h in range(HPC):
                    nc.vector.tensor_copy(
                        out=v_sb[:, kb, h, 0:DK],
                        in_=ps[:, h * DK:(h + 1) * DK])

            e_tiles = {}
            SPLIT_U = 7  # last unit: tiles cover [4 kb x 256 queries] so each
                         # query-quarter closes out while later quarters exp

            def qk_tile(u, t):
                """Scores^T tile [keys 128, 1024] + exp -> e (bf16)."""
                ih, h = divmod(u, HPC)
                mc, off = divmod(h, 2)
                off *= DK
                qk = qkp.tile([128, EXPW], f32, tag="qk", name="qk")
                if u == 0:
                    base, r = (0, t) if t < 8 else (4, t - 8)
                    qh, kbp = divmod(r, 4)
                    blocks = [(2 * (base + kbp) + j, qh * 512, j * 512, 512)
                              for j in range(2)]
                elif u == SPLIT_U:
                    qcc, half = divmod(t, 2)
                    blocks = [(8 * half + j, qcc * 128, j * 128, 128)
                              for j in range(8)]
                else:
                    blocks = [(t, ha * 512, ha * 512, 512) for ha in range(2)]
                for kb, qoff, coff, w in blocks:
                    nc.tensor.matmul(
                        qk[:, coff:coff + w],
                        lhsT=kT[mc][off:off + DK, kb * 128:(kb + 1) * 128],
                        rhs=qT[mc][off:off + DK,
                                   ih * IW + qoff: ih * IW + qoff + w],
                        start=True, stop=True)
                e = ep.tile([128, EXPW], bf16, tag="e", name=f"e{u}_{t}")
                nc.scalar.activation(out=e, in_=qk, func=AF.Exp,
                                     scale=1.0 / np.sqrt(DK))
                e_tiles[(u, t)] = e

            def e_slice(u, kb, qc):
                """lhsT slice [keys 128, queries 128] of unit u's e tiles."""
                if u == 0:
                    kbp = kb // 2
                    t = (qc // 4) * 4 + kbp if kbp < 4 \
                        else 8 + (qc // 4) * 4 + (kbp - 4)
                    off = (kb % 2) * 512 + (qc % 4) * 128
                elif u == SPLIT_U:
                    t = qc * 2 + kb // 8
                    off = (kb % 8) * 128
                else:
                    t, off = kb, qc * 128
                return e_tiles[(u, t)][:, off:off + 128]

            pv_psum = {}

            def pv_qc(u, qc):
                """ctx[q 128, dk+1] for queries qc of unit u; accumulate all kb.
                Then normalize into stage (per-partition scalar multiply)."""
                ih, h = divmod(u, HPC)
                qg, q4 = divmod(qc, 4)
                if q4 == 0:
                    pv_psum[(u, qg)] = pvp.tile([128, 4, DK + 1], f32,
                                                tag="pv", name="pv")
                pv = pv_psum[(u, qg)]
                for kb in range(NKB):
                    nc.tensor.matmul(
                        pv[:, q4, :],
                        lhsT=e_slice(u, kb, qc),
                        rhs=v_sb[:, kb, h, :],
                        start=(kb == 0), stop=(kb == NKB - 1))
                inv = invp.tile([128, 1], f32, tag="inv", name="inv")
                nc.vector.reciprocal(out=inv, in_=pv[:, q4, DK:DK + 1])
                nc.vector.tensor_scalar_mul(
                    out=stage[ih][qc][:, h * DK:(h + 1) * DK],
                    in0=pv[:, q4, 0:DK], scalar1=inv)

            def t_qc(ih, qc, mc):
                """Transpose normalized ctx block [q 128, m 128] -> ctx_t."""
                ps = psp.tile([128, 512], bf16, tag="ps", name="tp")
                nc.tensor.transpose(
                    ps[:, 0:128],
                    in_=stage[ih][qc][:, mc * 128:(mc + 1) * 128],
                    identity=ident)
                nc.vector.tensor_copy(
                    out=ctx_t[ih][mc][:, qc * 128:(qc + 1) * 128],
                    in_=ps[:, 0:128])

            def wo_qc(ih, qc):
                """Output projection for query chunk qc of half ih."""
                for nh in range(2):
                    ps = psp.tile([128, 512], f32, tag="ps", name="wops")
                    for mc in range(NMC):
                        nc.tensor.matmul(
                            ps,
                            lhsT=ctx_t[ih][mc][:, qc * 128:(qc + 1) * 128],
                            rhs=wo_sb[:, mc, nh * 512:(nh + 1) * 512],
                            start=(mc == 0), stop=(mc == NMC - 1))
                    os_ = ostp.tile([128, 512], f32, tag="ost", name="ost")
                    nc.vector.tensor_copy(out=os_, in_=ps)
                    nc.sync.dma_start(
                        out=out_d[(ih * 8 + qc) * 128:(ih * 8 + qc + 1) * 128,
                                  nh * 512:(nh + 1) * 512],
                        in_=os_)

            # ---- static filler schedule ----
            PK = lambda tens, sh, mc, sc: (lambda: proj_mk(tens, sh, mc, sc))
            PJV = lambda kb: (lambda: proj_v(kb))
            PV = lambda u, qc: (lambda: pv_qc(u, qc))
            T = lambda ih, mc: (lambda: [t_qc(ih, qc, mc) for qc in range(8)])
            T1q = lambda qc: (lambda: t_qc(1, qc, 1))
            WO = lambda ih, qc: (lambda: wo_qc(ih, qc))

            FILL = {
                (0, 0): [PK("k", 0, 0, 1)],
                (0, 1): [PK("q", 0, 0, 1)],
                (0, 3): [PK("k", 1, 0, 0)],
                (0, 5): [PK("k", 1, 0, 1)],
                (0, 7): [PK("k", 0, 1, 0)],
                (0, 9): [PK("k", 0, 1, 1)],
                (0, 11): [PK("q", 0, 1, 0)],
                (0, 13): [PK("q", 0, 1, 1)],
                (0, 15): [PK("k", 1, 1, 0)],
                (1, 0): [PJV(0)], (1, 2): [PJV(1)], (1, 3): [PJV(2)],
                (1, 5): [PJV(3)], (1, 6): [PJV(4)], (1, 8): [PJV(5)],
                (1, 9): [PJV(6)], (1, 11): [PJV(7)], (1, 12): [PJV(8)],
                (1, 14): [PJV(9)], (1, 15): [PJV(10)],
                (2, 0): [PJV(11)], (2, 1): [PK("k", 1, 1, 1)],
                (2, 2): [PJV(12)], (2, 3): [PJV(13)], (2, 4): [PJV(14)],
                (2, 5): [PJV(15)],
                (2, 6): [PV(0, 0)], (2, 7): [PV(0, 1)], (2, 8): [PV(0, 2)],
                (2, 9): [PV(0, 3)], (2, 10): [PV(0, 4)], (2, 11): [PV(0, 5)],
                (2, 12): [PV(0, 6)], (2, 13): [PV(0, 7)],
                (3, 0): [PV(1, 0)], (3, 1): [PV(1, 1)], (3, 2): [PV(1, 2)],
                (3, 3): [PV(1, 3)], (3, 4): [PV(1, 4)], (3, 5): [PV(1, 5)],
                (3, 6): [PV(1, 6)], (3, 7): [PV(1, 7)],
                (3, 8): [PK("q", 1, 0, 0)], (3, 10): [PK("q", 1, 0, 1)],
                (3, 12): [PK("q", 1, 1, 0)], (3, 14): [PK("q", 1, 1, 1)],
                (4, 0): [PV(2, 0)], (4, 1): [PV(2, 1)], (4, 2): [PV(2, 2)],
                (4, 3): [PV(2, 3)], (4, 4): [PV(2, 4)], (4, 5): [PV(2, 5)],
                (4, 6): [PV(2, 6)], (4, 7): [PV(2, 7)],
                (4, 8): [PV(3, 0)], (4, 9): [PV(3, 1)], (4, 10): [PV(3, 2)],
                (4, 11): [PV(3, 3)], (4, 12): [PV(3, 4)], (4, 13): [PV(3, 5)],
                (4, 14): [PV(3, 6)], (4, 15): [PV(3, 7)],
                (5, 0): [T(0, 0)], (5, 1): [T(0, 1)],
                (5, 2): [WO(0, 0)], (5, 4): [WO(0, 1)], (5, 6): [WO(0, 2)],
                (5, 8): [WO(0, 3)], (5, 10): [WO(0, 4)], (5, 12): [WO(0, 5)],
                (5, 14): [WO(0, 6)], (5, 15): [WO(0, 7)],
                (6, 0): [PV(4, 0)], (6, 1): [PV(4, 1)], (6, 2): [PV(4, 2)],
                (6, 3): [PV(4, 3)], (6, 4): [PV(4, 4)], (6, 5): [PV(4, 5)],
                (6, 6): [PV(4, 6)], (6, 7): [PV(4, 7)],
                (6, 8): [PV(5, 0)], (6, 9): [PV(5, 1)], (6, 10): [PV(5, 2)],
                (6, 11): [PV(5, 3)], (6, 12): [PV(5, 4)], (6, 13): [PV(5, 5)],
                (6, 14): [PV(5, 6)], (6, 15): [PV(5, 7)],
                (7, 0): [T(1, 0)],
                (7, 1): [PV(6, 0)], (7, 2): [PV(6, 1)],
                (7, 3): [PV(7, 0), PV(6, 2)],
                (7, 4): [T1q(0), PV(6, 3)],
                (7, 5): [PV(7, 1), WO(1, 0)],
                (7, 6): [T1q(1), PV(6, 4)],
                (7, 7): [PV(7, 2), WO(1, 1)],
                (7, 8): [T1q(2), PV(6, 5)],
                (7, 9): [PV(7, 3), WO(1, 2)],
                (7, 10): [T1q(3), PV(6, 6)],
                (7, 11): [PV(7, 4), WO(1, 3)],
                (7, 12): [T1q(4), PV(6, 7)],
                (7, 13): [PV(7, 5), WO(1, 4)],
                (7, 14): [T1q(5), PV(7, 6)],
                (7, 15): [WO(1, 5), T1q(6)],
            }

            def proj_mk_pair(tens, sh, mc):
                """Both 512-col groups of a projection, dc-major interleaved
                so the last matmuls land right as the final x chunk arrives."""
                w_sb = wq_sb if tens == "q" else wk_sb
                pss = [psp.tile([128, 512], f32, tag="ps", name="ps")
                       for _ in range(2)]
                for dc in range(NDC):
                    for sc in range(2):
                        nc.tensor.matmul(
                            pss[sc],
                            lhsT=w_sb[:, dc, mc * 128:(mc + 1) * 128],
                            rhs=xc[(tens, sh, sc)][:, dc, :],
                            start=(dc == 0), stop=(dc == NDC - 1))
                for sc in range(2):
                    dst = (qT if tens == "q" else kT)[mc][
                        :, sh * 1024 + sc * 512: sh * 1024 + (sc + 1) * 512]
                    if tens == "q":
                        nc.vector.tensor_scalar_add(
                            out=dst, in0=pss[sc], scalar1=bq_sb[:, mc:mc + 1])
                    else:
                        nc.vector.tensor_copy(out=dst, in_=pss[sc])

            # ---- main pipeline ----
            # prologue: first projections (DMA-gated)
            proj_mk("k", 0, 0, 0)
            proj_mk("q", 0, 0, 0)
            # 8 attention units paced by the exp stream
            for u in range(8):
                for t in range(NT):
                    qk_tile(u, t)
                    for fn in FILL.get((u, t), []):
                        fn()
            # tail: only the final query-chunk's chain remains
            pv_qc(7, 7)
            wo_qc(1, 6)
            t_qc(1, 7, 1)
            wo_qc(1, 7)

    nc.compile()
    return nc


def _get_nc(debug=False):
    key = ("nc", debug)
    if key not in _cached:
        _cached[key] = _build(debug)
    return _cached[key]


def _get_runner():
    """Build (once) a jitted 8-core SPMD executable mirroring
    bass2jax.run_bass_via_pjrt, reusable across calls for benchmarking."""
    if "runner" in _cached:
        return _cached["runner"]
    import jax
    import jax.numpy as jnp
    from jax.experimental.shard_map import shard_map
    from jax.sharding import Mesh, PartitionSpec
    import concourse.mybir as mybir
    from concourse import bass2jax

    bass2jax.install_neuronx_cc_hook()
    nc = _get_nc()
    assert nc.dbg_addr is None
    partition_name = nc.partition_id_tensor.name if nc.partition_id_tensor else None

    in_names, out_names, out_avals, zero_outs = [], [], [], []
    for alloc in nc.m.functions[0].allocations:
        if not isinstance(alloc, mybir.MemoryLocationSet):
            continue
        name = alloc.memorylocations[0].name
        if alloc.kind == "ExternalInput":
            if name != partition_name:
                in_names.append(name)
        elif alloc.kind == "ExternalOutput":
            out_names.append(name)
            shape = tuple(alloc.tensor_shape)
            dtype = mybir.dt.np(alloc.dtype)
            out_avals.append(jax.core.ShapedArray(shape, dtype))
            zero_outs.append(np.zeros(shape, dtype))
    n_params = len(in_names)
    all_in_names = in_names + out_names
    if partition_name is not None:
        all_in_names = all_in_names + [partition_name]
    donate = tuple(range(n_params, n_params + len(out_names)))

    def _body(*args):
        operands = list(args)
        if partition_name is not None:
            operands.append(bass2jax.partition_id_tensor())
        outs = bass2jax._bass_exec_p.bind(
            *operands,
            out_avals=tuple(out_avals),
            in_names=tuple(all_in_names),
            out_names=tuple(out_names),
            lowering_input_output_aliases=(),
            sim_require_finite=True,
            sim_require_nnan=True,
            nc=nc,
        )
        return tuple(outs)

    devices = jax.devices()[:NC]
    mesh = Mesh(np.asarray(devices), ("core",))
    nin = n_params + len(out_names)
    sharded = jax.jit(
        shard_map(
            _body,
            mesh=mesh,
            in_specs=(PartitionSpec("core"),) * nin,
            out_specs=(PartitionSpec("core"),) * len(out_names),
            check_rep=False,
        ),
        donate_argnums=donate,
        keep_unused=True,
    )

    def run(in_maps):
        concat_in = [
            np.concatenate([np.asarray(in_maps[c][n]) for c in range(NC)], axis=0)
            for n in in_names
        ]
        concat_zeros = [
            np.zeros((NC * z.shape[0], *z.shape[1:]), z.dtype) for z in zero_outs
        ]
        out_arrs = sharded(*concat_in, *concat_zeros)
        return [
            {
                n: np.asarray(out_arrs[i]).reshape(NC, *out_avals[i].shape)[c]
                for i, n in enumerate(out_names)
            }
            for c in range(NC)
        ]

    _cached["runner"] = (run, sharded, in_names, out_names, out_avals, zero_outs)
    return _cached["runner"]


def _make_in_maps(query, key, value, Wq, bq, Wk, bk, Wv, bv, Wo, bo):
    import ml_dtypes
    bf16 = ml_dtypes.bfloat16

    query = np.asarray(query, dtype=np.float32)
    key = np.asarray(key, dtype=np.float32)
    value = np.asarray(value, dtype=np.float32)
    Wq, Wk, Wv, Wo = (np.asarray(a, dtype=np.float32) for a in (Wq, Wk, Wv, Wo))
    bq = np.asarray(bq, dtype=np.float32)
    B = query.shape[0]
    ident = np.eye(128, dtype=bf16)

    xqT = [np.ascontiguousarray(query[b].T).astype(bf16) for b in range(B)]
    xkT = [np.ascontiguousarray(key[b].T).astype(bf16) for b in range(B)]
    xvT = [np.ascontiguousarray(value[b].T).astype(bf16) for b in range(B)]

    in_maps = []
    for c in range(NC):
        b, hg = divmod(c, NC // B)
        sl = slice(hg * M, (hg + 1) * M)
        in_maps.append(
            {
                "xqT": xqT[b],
                "xkT": xkT[b],
                "xvT": xvT[b],
                "wq": np.ascontiguousarray(Wq[:, sl]).astype(bf16),
                "wk": np.ascontiguousarray(Wk[:, sl]).astype(bf16),
                "wv": np.ascontiguousarray(Wv[:, sl]).astype(bf16),
                "wo": np.ascontiguousarray(Wo[sl, :]).astype(bf16),
                "bq": np.ascontiguousarray(bq[sl]),
                "ident": ident,
            }
        )
    return in_maps


def kernel(query, key, value, Wq, bq, Wk, bk, Wv, bv, Wo, bo):
    in_maps = _make_in_maps(query, key, value, Wq, bq, Wk, bk, Wv, bv, Wo, bo)
    run = _get_runner()[0]
    results = run(in_maps)

    B = np.asarray(query).shape[0]
    bo = np.asarray(bo, dtype=np.float32)
    bv = np.asarray(bv, dtype=np.float32)
    Wo_f = np.asarray(Wo, dtype=np.float32)
    base = bo + bv @ Wo_f  # bv contributes exactly bv @ Wo (sum of attn = 1)
    full = np.zeros((B, S, D), np.float32)
    for b in range(B):
        acc = np.zeros((S, D), np.float32)
        for g in range(NC // B):
            acc += results[b * (NC // B) + g]["out"]
        full[b] = acc + base[None, :]
    return full
